# revision 13
# baseline (speedup 1.0000x reference)
"""Trainium2 Bass kernel for the 2-layer GAT + mean-pool + MLP head problem.

Strategy (8-core SPMD, single NEFF):
  - Nodes are sharded by destination across 8 cores (6250 each, padded 6272).
    Per-core local node l -> (block t = l % 49, lane p = l // 49); padded node
    table row r = core*6272 + p*49 + t so the SBUF->DRAM table write is
    contiguous per partition.
  - Per layer: each core computes an fp16 "aug" row [h | asrc | adst] (144
    cols) for its own nodes with one matmul per block (lhsT = x^T tile,
    rhs = [W | W@Asrc_bd | W@Adst_bd]); AllGather builds the full 50176-row
    gather table in每 core's HBM.
  - Edge phase: edges (with self-loops) are sorted by dst block and padded to
    T tiles of 128 edges per block (T = global max, identical program on all
    cores).  For batches of U tiles one indirect DMA gathers 128*U src rows
    (288B each) and a second cheap indirect DMA gathers the 16B adst slices
    by dst.  ex = exp(max(z, 0.2z)) with z = asrc+adst; h_scaled = h*ex
    (broadcast per head); a one-hot [128e,128d] built by is_equal against an
    iota constant feeds matmul psum += onehot^T @ [h_scaled | ex], giving the
    unnormalized aggregation and the softmax denominators in one pass.
  - Block epilogue: out = psum[:, :128] * (1/max(s,1e-30)) per head, + bias,
    ELU (= max(x,0) + min(exp(x)-1, 0)); layer 1 feeds a PE transpose +
    matmul producing the next layer's aug rows; layer 2 feeds the
    graph-mean-pool matmul (host-built graph one-hot).
  - Pool partials are AllReduced (32KB), then every core runs the tiny MLP +
    log_softmax redundantly; core 0's [64,10] outputs are returned.

kernel(**inputs) takes the FULL unsharded inputs and returns
(log_softmax(logits), logits) like the reference.
"""

import numpy as np

import concourse.bass as bass
import concourse.mybir as mybir
import concourse.tile as tile
from concourse import bacc
from concourse.bass import IndirectOffsetOnAxis
from concourse.bass_utils import run_bass_kernel_spmd

F16 = mybir.dt.float16
F32 = mybir.dt.float32
I32 = mybir.dt.int32
AX = mybir.AluOpType

NCORES = 8


def gat_config(N=50000, E=800000, F=128, H=8, C=16, G=64, NCLS=10, U=32):
    NPC = N // NCORES
    BLOCKS = (NPC + 127) // 128
    NPAD = BLOCKS * 128
    return dict(N=N, E=E, F=F, H=H, C=C, G=G, NCLS=NCLS, U=U, NPC=NPC,
                BLOCKS=BLOCKS, NPAD=NPAD, TBLROWS=NCORES * NPAD, AUGW=F + 2 * H)


def _blockdiag(a, H, C):
    m = np.zeros((H * C, H), np.float32)
    for h in range(H):
        m[h * C:(h + 1) * C, h] = a[h]
    return m


def host_prep(inputs, cfg):
    """Builds per-core device input dicts + meta. Pure index/layout work."""
    N, E, F, H, C, G = cfg["N"], cfg["E"], cfg["F"], cfg["H"], cfg["C"], cfg["G"]
    NPC, BLOCKS, NPAD = cfg["NPC"], cfg["BLOCKS"], cfg["NPAD"]
    AUGW = cfg["AUGW"]

    x = np.asarray(inputs["x"], np.float32)
    ei = np.asarray(inputs["edge_index"], np.int64)
    batch = np.asarray(inputs["batch"], np.int64)

    W1 = np.asarray(inputs["W1"], np.float32)
    W2 = np.asarray(inputs["W2"], np.float32)
    w1aug = np.concatenate(
        [W1, W1 @ _blockdiag(np.asarray(inputs["a_src1"], np.float32), H, C),
         W1 @ _blockdiag(np.asarray(inputs["a_dst1"], np.float32), H, C)], 1)
    w2aug = np.concatenate(
        [W2, W2 @ _blockdiag(np.asarray(inputs["a_src2"], np.float32), H, C),
         W2 @ _blockdiag(np.asarray(inputs["a_dst2"], np.float32), H, C)], 1)

    src = np.concatenate([ei[0], np.arange(N, dtype=np.int64)])
    dst = np.concatenate([ei[1], np.arange(N, dtype=np.int64)])

    core = dst // NPC
    loc = dst - core * NPC
    t_blk = loc % BLOCKS
    p_lane = loc // BLOCKS

    def g2r(g):
        c = g // NPC
        l = g - c * NPC
        return (c * NPAD + (l // BLOCKS) * BLOCKS + (l % BLOCKS)).astype(np.int32)

    key = (core * BLOCKS + t_blk).astype(np.int64)
    order = np.argsort(key, kind="stable")
    counts = np.bincount(key, minlength=NCORES * BLOCKS)
    T = int(np.ceil(counts.max() / 128))
    NT = BLOCKS * T
    EPB = T * 128

    src_rows = g2r(src[order])
    dst_rows = g2r(dst[order])
    p_s = p_lane[order]

    srcR = np.zeros((NCORES, NT * 128), np.int32)
    dstR = np.zeros((NCORES, NT * 128), np.int32)
    dstloc = np.full((NCORES, NT * 128), 200.0, np.float16)
    ofs = np.concatenate([[0], np.cumsum(counts)])
    for c in range(NCORES):
        for b in range(BLOCKS):
            k = c * BLOCKS + b
            cnt = counts[k]
            sl = slice(ofs[k], ofs[k + 1])
            srcR[c, b * EPB:b * EPB + cnt] = src_rows[sl]
            dstR[c, b * EPB:b * EPB + cnt] = dst_rows[sl]
            dstloc[c, b * EPB:b * EPB + cnt] = p_s[sl].astype(np.float16)
    srcT = np.ascontiguousarray(srcR.reshape(NCORES, NT, 128).transpose(0, 2, 1))
    dstgT = np.ascontiguousarray(dstR.reshape(NCORES, NT, 128).transpose(0, 2, 1))
    dstlocT = np.ascontiguousarray(dstloc.reshape(NCORES, NT, 128).transpose(0, 2, 1))

    # x^T per core in (t,p) column order: col t*128+p <- global node c*NPC + p*BLOCKS + t
    tt = np.arange(NPAD) // 128
    pp = np.arange(NPAD) % 128
    l_of_col = pp * BLOCKS + tt
    xt = np.zeros((NCORES, F, NPAD), np.float16)
    for c in range(NCORES):
        ok = l_of_col < NPC
        cols = np.where(ok, c * NPC + np.minimum(l_of_col, NPC - 1), 0)
        xr = np.where(ok[:, None], x[cols], 0.0)
        xt[c] = xr.T.astype(np.float16)

    # graph one-hot for pooling: gone[c, p, t*G+g]
    gone = np.zeros((NCORES, 128, BLOCKS * G), np.float16)
    for c in range(NCORES):
        l = pp * BLOCKS + tt  # same enumeration
        ok = l < NPC
        gids = batch[np.where(ok, c * NPC + np.minimum(l, NPC - 1), 0)]
        for col in range(NPAD):
            if ok[col]:
                gone[c, pp[col], tt[col] * G + int(gids[col])] = 1.0
    cnt = np.bincount(batch, minlength=G).astype(np.float32)
    inv_cnt = (1.0 / np.maximum(cnt, 1.0)).astype(np.float32).reshape(G, 1)

    U = min(cfg["U"], NT)
    iota = np.tile(np.arange(128, dtype=np.float16), U)
    iota_rep = np.broadcast_to(iota, (128, U * 128)).copy()

    ident_h = np.eye(128, dtype=np.float16)
    ident_f = np.eye(64, dtype=np.float32)

    b1 = np.asarray(inputs["b1"], np.float32)
    b2 = np.asarray(inputs["b2"], np.float32)
    l1b = np.asarray(inputs["lin1_b"], np.float32)
    l2b = np.asarray(inputs["lin2_b"], np.float32)
    meta = dict(cfg, T=T, NT=NT, U=U,
                bias1=bool(np.any(b1 != 0)), bias2=bool(np.any(b2 != 0)),
                lbias1=bool(np.any(l1b != 0)), lbias2=bool(np.any(l2b != 0)))

    common = dict(
        w1aug=w1aug.astype(np.float16), w2aug=w2aug.astype(np.float16),
        iota_rep=iota_rep, ident_h=ident_h, ident_f=ident_f,
        lin1w=np.asarray(inputs["lin1_W"], np.float32),
        lin2w=np.asarray(inputs["lin2_W"], np.float32),
        inv_cnt=inv_cnt,
    )
    if meta["bias1"]:
        common["b1rep"] = np.broadcast_to(b1.astype(np.float32), (128, F)).copy()
    if meta["bias2"]:
        common["b2rep"] = np.broadcast_to(b2.astype(np.float32), (128, F)).copy()
    if meta["lbias1"]:
        common["l1brep"] = np.broadcast_to(l1b, (cfg["G"], l1b.shape[0])).copy()
    if meta["lbias2"]:
        common["l2brep"] = np.broadcast_to(l2b, (cfg["G"], l2b.shape[0])).copy()

    in_maps = []
    for c in range(NCORES):
        m = dict(common)
        m["xt_loc"] = xt[c]
        m["srcT"] = srcT[c]
        m["dstgT"] = dstgT[c]
        m["dstlocT"] = dstlocT[c]
        m["gone"] = gone[c]
        in_maps.append(m)
    return meta, in_maps


def build_nc(meta):
    F, H, C, G, NCLS = meta["F"], meta["H"], meta["C"], meta["G"], meta["NCLS"]
    BLOCKS, NPAD, TBLROWS = meta["BLOCKS"], meta["NPAD"], meta["TBLROWS"]
    T, NT, U, AUGW = meta["T"], meta["NT"], meta["U"], meta["AUGW"]
    HC = H * C  # == F
    REPW = F + H  # matmul rhs width: [h_scaled | ex]

    nc = bacc.Bacc("TRN2", target_bir_lowering=False, debug=False,
                   num_devices=NCORES)

    # --- I/O ---
    d_xt = nc.dram_tensor("xt_loc", [F, NPAD], F16, kind="ExternalInput")
    d_src = nc.dram_tensor("srcT", [128, NT], I32, kind="ExternalInput")
    d_dstg = nc.dram_tensor("dstgT", [128, NT], I32, kind="ExternalInput")
    d_dstl = nc.dram_tensor("dstlocT", [128, NT], F16, kind="ExternalInput")
    d_gone = nc.dram_tensor("gone", [128, BLOCKS * G], F16, kind="ExternalInput")
    d_w1 = nc.dram_tensor("w1aug", [F, AUGW], F16, kind="ExternalInput")
    d_w2 = nc.dram_tensor("w2aug", [F, AUGW], F16, kind="ExternalInput")
    d_iota = nc.dram_tensor("iota_rep", [128, U * 128], F16, kind="ExternalInput")
    d_idh = nc.dram_tensor("ident_h", [128, 128], F16, kind="ExternalInput")
    d_idf = nc.dram_tensor("ident_f", [64, 64], F32, kind="ExternalInput")
    d_l1w = nc.dram_tensor("lin1w", [F, C], F32, kind="ExternalInput")
    d_l2w = nc.dram_tensor("lin2w", [C, NCLS], F32, kind="ExternalInput")
    d_icnt = nc.dram_tensor("inv_cnt", [G, 1], F32, kind="ExternalInput")
    d_b1 = (nc.dram_tensor("b1rep", [128, F], F32, kind="ExternalInput")
            if meta["bias1"] else None)
    d_b2 = (nc.dram_tensor("b2rep", [128, F], F32, kind="ExternalInput")
            if meta["bias2"] else None)
    d_l1b = (nc.dram_tensor("l1brep", [G, C], F32, kind="ExternalInput")
             if meta["lbias1"] else None)
    d_l2b = (nc.dram_tensor("l2brep", [G, NCLS], F32, kind="ExternalInput")
             if meta["lbias2"] else None)
    d_lsm = nc.dram_tensor("out_lsm", [G, NCLS], F32, kind="ExternalOutput")
    d_logit = nc.dram_tensor("out_logits", [G, NCLS], F32, kind="ExternalOutput")
    dbg = meta.get("dbg", False)
    if dbg:
        d_dbg_aug = nc.dram_tensor("dbg_aug", [128, BLOCKS * AUGW], F16,
                                   kind="ExternalOutput")
        d_dbg_tbl = nc.dram_tensor("dbg_tbl", [128, AUGW], F16,
                                   kind="ExternalOutput")
        d_dbg_ps = nc.dram_tensor("dbg_ps", [128, F + H], F32,
                                  kind="ExternalOutput")
        d_dbg_eo = nc.dram_tensor("dbg_eo", [128, F], F16,
                                  kind="ExternalOutput")
        d_dbg_pool = nc.dram_tensor("dbg_pool", [G, F], F32,
                                    kind="ExternalOutput")
        d_dbg_g = nc.dram_tensor("dbg_g", [128, AUGW], F16,
                                 kind="ExternalOutput")
        d_dbg_ex = nc.dram_tensor("dbg_ex", [128, H], F16,
                                  kind="ExternalOutput")

    # --- internal DRAM (collectives) ---
    aug_loc = [nc.dram_tensor(f"aug_loc{i}", [NPAD, AUGW], F16) for i in (1, 2)]
    table = [nc.dram_tensor(f"table{i}", [TBLROWS, AUGW], F16, addr_space="Shared")
             for i in (1, 2)]
    pool_part = nc.dram_tensor("pool_part", [G, F], F32)
    pool_full = nc.dram_tensor("pool_full", [G, F], F32, addr_space="Shared")
    RG = [list(range(NCORES))]

    from contextlib import ExitStack
    with tile.TileContext(nc) as tc, ExitStack() as ctx:
        cpool = ctx.enter_context(tc.tile_pool(name="consts", bufs=1))
        gpool = ctx.enter_context(tc.tile_pool(name="gath", bufs=2))
        hpool = ctx.enter_context(tc.tile_pool(name="hsex", bufs=2))
        opool = ctx.enter_context(tc.tile_pool(name="oneh", bufs=2))
        zpool = ctx.enter_context(tc.tile_pool(name="zl", bufs=3))
        apool = ctx.enter_context(tc.tile_pool(name="adL", bufs=2))
        epool = ctx.enter_context(tc.tile_pool(name="epi", bufs=3))
        augp = ctx.enter_context(tc.tile_pool(name="augsb", bufs=2))
        psp = ctx.enter_context(tc.tile_pool(name="ps", bufs=3, space="PSUM"))
        pst = ctx.enter_context(tc.tile_pool(name="pst", bufs=2, space="PSUM"))
        psa = ctx.enter_context(tc.tile_pool(name="psa", bufs=2, space="PSUM"))
        psg = ctx.enter_context(tc.tile_pool(name="psg", bufs=1, space="PSUM"))

        def load_const(dram, shape, dtype):
            t = cpool.tile(shape, dtype, tag=dram.name)
            nc.sync.dma_start(out=t[:], in_=dram[:])
            return t

        xt_sb = load_const(d_xt, [F, NPAD], F16)
        src_sb = load_const(d_src, [128, NT], I32)
        dstg_sb = load_const(d_dstg, [128, NT], I32)
        dstl_sb = load_const(d_dstl, [128, NT], F16)
        gone_sb = load_const(d_gone, [128, BLOCKS * G], F16)
        w1_sb = load_const(d_w1, [F, AUGW], F16)
        w2_sb = load_const(d_w2, [F, AUGW], F16)
        iota_sb = load_const(d_iota, [128, U * 128], F16)
        idh_sb = load_const(d_idh, [128, 128], F16)
        idf_sb = load_const(d_idf, [64, 64], F32)
        l1w_sb = load_const(d_l1w, [F, C], F32)
        l2w_sb = load_const(d_l2w, [C, NCLS], F32)
        icnt_sb = load_const(d_icnt, [G, 1], F32)
        b1_sb = load_const(d_b1, [128, F], F32) if d_b1 is not None else None
        b2_sb = load_const(d_b2, [128, F], F32) if d_b2 is not None else None
        l1b_sb = load_const(d_l1b, [G, C], F32) if d_l1b is not None else None
        l2b_sb = load_const(d_l2b, [G, NCLS], F32) if d_l2b is not None else None

        def build_aug_from_xt(w_sb):
            """aug rows for own nodes from resident x^T; returns sbuf tile."""
            aug_sb = augp.tile([128, BLOCKS * AUGW], F16, tag="augsb")
            for t in range(BLOCKS):
                ps = psa.tile([128, AUGW], F32, tag="psaug")
                nc.tensor.matmul(out=ps[:], lhsT=xt_sb[:, t * 128:(t + 1) * 128],
                                 rhs=w_sb[:], start=True, stop=True)
                nc.vector.tensor_copy(out=aug_sb[:, t * AUGW:(t + 1) * AUGW],
                                      in_=ps[:])
            return aug_sb

        def publish_table(aug_sb, which):
            dst = aug_loc[which]
            # DRAM rows r = p*BLOCKS + t  <=> view [(p t), f] -> [p, (t f)]
            nc.sync.dma_start(
                out=dst[:, :].rearrange("(p t) f -> p (t f)", t=BLOCKS),
                in_=aug_sb[:])
            nc.gpsimd.collective_compute(
                "AllGather", AX.bypass, replica_groups=RG,
                ins=[dst[:, :].opt()], outs=[table[which][:, :].opt()])

        def elu_inplace(v_sb, width, out_tile):
            """out_tile(fp16) = elu(v_sb) = max(v,0) + min(exp(v)-1, 0)."""
            t_sb = epool.tile([128, width], F32, tag="elu_t")
            nc.scalar.activation(out=t_sb[:], in_=v_sb[:],
                                 func=mybir.ActivationFunctionType.Exp)
            nc.vector.tensor_scalar(out=t_sb[:], in0=t_sb[:], scalar1=1.0,
                                    scalar2=0.0, op0=AX.subtract, op1=AX.min)
            nc.vector.scalar_tensor_tensor(out=out_tile[:], in0=v_sb[:],
                                           scalar=0.0, op0=AX.max,
                                           in1=t_sb[:], op1=AX.add)

        def edge_phase(layer):
            """layer 0: consumes table[0], produces aug_sb for table[1].
               layer 1: consumes table[1], accumulates pool psum. Returns
               aug_sb (layer 0) or pool psum tile (layer 1)."""
            tbl = table[layer]
            bias_sb = (b1_sb, b2_sb)[layer]
            if layer == 0:
                out_aug = augp.tile([128, BLOCKS * AUGW], F16, tag="augsb")
            else:
                pool_ps = psg.tile([G, F], F32, tag="poolps")

            # adst stream gather: the HW indirect-DMA lowering supports ONE
            # index per partition per instruction, so gather per tile.
            adL = apool.tile([128, NT * H], F16, tag="adL")
            for t in range(NT):
                nc.gpsimd.indirect_dma_start(
                    out=adL[:, t * H:(t + 1) * H], out_offset=None,
                    in_=tbl[:, :],
                    in_offset=IndirectOffsetOnAxis(ap=dstg_sb[:, t:t + 1], axis=0),
                    element_offset=F + H)

            nbatch = (NT + U - 1) // U
            ps_cur = None
            for bi in range(nbatch):
                u0 = bi * U
                ub = min(U, NT - u0)
                g_sb = gpool.tile([128, U * AUGW], F16, tag="g")
                for u in range(ub):
                    nc.gpsimd.indirect_dma_start(
                        out=g_sb[:, u * AUGW:(u + 1) * AUGW], out_offset=None,
                        in_=tbl[:, :],
                        in_offset=IndirectOffsetOnAxis(
                            ap=src_sb[:, u0 + u:u0 + u + 1], axis=0))
                g3 = g_sb[:, :ub * AUGW].rearrange("p (u f) -> p u f", f=AUGW)
                if dbg and layer == 0 and bi == 0:
                    nc.sync.dma_start(out=d_dbg_g[:, :], in_=g_sb[:, :AUGW])

                # z = asrc[src] + adst[dst]; leaky via max(z, 0.2z); ex = exp
                zl = zpool.tile([128, U * H], F16, tag="zl")
                nc.vector.tensor_tensor(
                    out=zl[:, :ub * H], in0=g3[:, :, F:F + H],
                    in1=adL[:, u0 * H:(u0 + ub) * H], op=AX.add)
                zv = zl[:, :ub * H]
                nc.vector.scalar_tensor_tensor(
                    out=zv, in0=zv, scalar=0.2, op0=AX.mult, in1=zv, op1=AX.max)

                he = hpool.tile([128, U * REPW], F16, tag="he")
                he3 = he[:, :ub * REPW].rearrange("p (u f) -> p u f", f=REPW)
                nc.scalar.activation(
                    out=he3[:, :, F:F + H],
                    in_=zl[:, :ub * H].rearrange("p (u h) -> p u h", h=H),
                    func=mybir.ActivationFunctionType.Exp)
                # h_scaled = h * ex (broadcast over C channels per head)
                nc.vector.tensor_tensor(
                    out=he3[:, :, 0:F].rearrange("p u (h c) -> p u h c", c=C),
                    in0=g3[:, :, 0:F].rearrange("p u (h c) -> p u h c", c=C),
                    in1=he3[:, :, F:F + H].to_broadcast([128, ub, H, C]),
                    op=AX.mult)

                oh = opool.tile([128, U * 128], F16, tag="oh")
                nc.vector.tensor_tensor(
                    out=oh[:, :ub * 128].rearrange("p (u j) -> p u j", j=128),
                    in0=iota_sb[:, :ub * 128].rearrange("p (u j) -> p u j", j=128),
                    in1=dstl_sb[:, u0:u0 + ub].to_broadcast([128, ub, 128]),
                    op=AX.is_equal)

                for u in range(ub):
                    t = u0 + u
                    b, k = t // T, t % T
                    if k == 0:
                        ps_cur = psp.tile([128, REPW], F32, tag="psblk")
                    nc.tensor.matmul(
                        out=ps_cur[:], lhsT=oh[:, u * 128:(u + 1) * 128],
                        rhs=he[:, u * REPW:(u + 1) * REPW],
                        start=(k == 0), stop=(k == T - 1))
                    if k == T - 1:
                        # ---- block epilogue ----
                        if dbg and layer == 0 and b == 0:
                            ps_dbg = epool.tile([128, F + H], F32, tag="psdbg")
                            nc.vector.tensor_copy(out=ps_dbg[:], in_=ps_cur[:])
                            nc.sync.dma_start(out=d_dbg_ps[:, :], in_=ps_dbg[:])
                        s_sb = epool.tile([128, H], F32, tag="s")
                        nc.vector.tensor_scalar(out=s_sb[:], in0=ps_cur[:, F:F + H],
                                                scalar1=1e-30, scalar2=None,
                                                op0=AX.max)
                        r_sb = epool.tile([128, H], F32, tag="r")
                        nc.vector.reciprocal(out=r_sb[:], in_=s_sb[:])
                        v_sb = epool.tile([128, F], F32, tag="v")
                        nc.vector.tensor_tensor(
                            out=v_sb[:].rearrange("p (h c) -> p h c", c=C),
                            in0=ps_cur[:, 0:F].rearrange("p (h c) -> p h c", c=C),
                            in1=r_sb[:].to_broadcast([128, H, C]), op=AX.mult)
                        if bias_sb is not None:
                            nc.vector.tensor_tensor(out=v_sb[:], in0=v_sb[:],
                                                    in1=bias_sb[:], op=AX.add)
                        eo = epool.tile([128, F], F16, tag="eo")
                        elu_inplace(v_sb, F, eo)
                        if dbg and layer == 0 and b == 0:
                            nc.sync.dma_start(out=d_dbg_eo[:, :], in_=eo[:])
                            nc.sync.dma_start(
                                out=d_dbg_ex[:, :],
                                in_=he[:, u * REPW + F:u * REPW + F + H])
                        if layer == 0:
                            trp = pst.tile([128, 128], F16, tag="trps")
                            nc.tensor.transpose(out=trp[:], in_=eo[:],
                                                identity=idh_sb[:])
                            trs = epool.tile([128, 128], F16, tag="trsb")
                            nc.vector.tensor_copy(out=trs[:], in_=trp[:])
                            ap2 = psa.tile([128, AUGW], F32, tag="psaug")
                            nc.tensor.matmul(out=ap2[:], lhsT=trs[:],
                                             rhs=w2_sb[:], start=True, stop=True)
                            nc.vector.tensor_copy(
                                out=out_aug[:, b * AUGW:(b + 1) * AUGW],
                                in_=ap2[:])
                        else:
                            nc.tensor.matmul(
                                out=pool_ps[:],
                                lhsT=gone_sb[:, b * G:(b + 1) * G],
                                rhs=eo[:], start=(b == 0), stop=(b == BLOCKS - 1))
            return out_aug if layer == 0 else pool_ps

        # ---------------- pipeline ----------------
        aug1_sb = build_aug_from_xt(w1_sb)
        if dbg:
            nc.sync.dma_start(out=d_dbg_aug[:, :], in_=aug1_sb[:])
        publish_table(aug1_sb, 0)
        if dbg:
            tbl_dbg = epool.tile([128, AUGW], F16, tag="tbldbg")
            nc.sync.dma_start(out=tbl_dbg[:], in_=table[0][0:128, :])
            nc.sync.dma_start(out=d_dbg_tbl[:, :], in_=tbl_dbg[:])
        aug2_sb = edge_phase(0)
        publish_table(aug2_sb, 1)
        pool_ps = edge_phase(1)

        # pooling allreduce
        psum_sb = epool.tile([G, F], F32, tag="poolsb")
        nc.vector.tensor_copy(out=psum_sb[:], in_=pool_ps[:])
        if dbg:
            nc.sync.dma_start(out=d_dbg_pool[:, :], in_=psum_sb[:])
        nc.sync.dma_start(out=pool_part[:, :], in_=psum_sb[:])
        nc.gpsimd.collective_compute(
            "AllReduce", AX.add, replica_groups=RG,
            ins=[pool_part[:, :].opt()], outs=[pool_full[:, :].opt()])
        hg_sb = epool.tile([G, F], F32, tag="hg")
        nc.sync.dma_start(out=hg_sb[:], in_=pool_full[:, :])
        nc.vector.tensor_scalar(out=hg_sb[:], in0=hg_sb[:],
                                scalar1=icnt_sb[:, 0:1], scalar2=None,
                                op0=AX.mult)

        # MLP: z1 = elu(hg @ lin1W + b); logits = z1 @ lin2W + b
        hgT_ps = pst.tile([F, G], F32, tag="trps")
        nc.tensor.transpose(out=hgT_ps[:], in_=hg_sb[:], identity=idf_sb[:G, :G])
        hgT_sb = epool.tile([F, G], F32, tag="hgTs")
        nc.vector.tensor_copy(out=hgT_sb[:], in_=hgT_ps[:])
        z1_ps = psa.tile([G, C], F32, tag="psaug")
        nc.tensor.matmul(out=z1_ps[:], lhsT=hgT_sb[:], rhs=l1w_sb[:],
                         start=True, stop=True)
        z1_sb = epool.tile([G, C], F32, tag="z1s")
        if l1b_sb is not None:
            nc.vector.tensor_tensor(out=z1_sb[:], in0=z1_ps[:], in1=l1b_sb[:],
                                    op=AX.add)
        else:
            nc.vector.tensor_copy(out=z1_sb[:], in_=z1_ps[:])
        z1e_sb = epool.tile([G, C], F32, tag="z1e")
        t1 = epool.tile([G, C], F32, tag="t1")
        nc.scalar.activation(out=t1[:], in_=z1_sb[:],
                             func=mybir.ActivationFunctionType.Exp)
        nc.vector.tensor_scalar(out=t1[:], in0=t1[:], scalar1=1.0, scalar2=0.0,
                                op0=AX.subtract, op1=AX.min)
        nc.vector.scalar_tensor_tensor(out=z1e_sb[:], in0=z1_sb[:], scalar=0.0,
                                       op0=AX.max, in1=t1[:], op1=AX.add)
        z1T_ps = pst.tile([C, G], F32, tag="trps")
        nc.tensor.transpose(out=z1T_ps[:], in_=z1e_sb[:], identity=idf_sb[:G, :G])
        z1T_sb = epool.tile([C, G], F32, tag="z1Ts")
        nc.vector.tensor_copy(out=z1T_sb[:], in_=z1T_ps[:])
        lg_ps = psa.tile([G, NCLS], F32, tag="psaug")
        nc.tensor.matmul(out=lg_ps[:], lhsT=z1T_sb[:], rhs=l2w_sb[:],
                         start=True, stop=True)
        lg_sb = epool.tile([G, NCLS], F32, tag="lgs")
        if l2b_sb is not None:
            nc.vector.tensor_tensor(out=lg_sb[:], in0=lg_ps[:], in1=l2b_sb[:],
                                    op=AX.add)
        else:
            nc.vector.tensor_copy(out=lg_sb[:], in_=lg_ps[:])

        # log_softmax
        m_sb = epool.tile([G, 1], F32, tag="m")
        nc.vector.tensor_reduce(out=m_sb[:], in_=lg_sb[:],
                                axis=mybir.AxisListType.X, op=AX.max)
        nm_sb = epool.tile([G, 1], F32, tag="nm")
        nc.vector.tensor_scalar(out=nm_sb[:], in0=m_sb[:], scalar1=-1.0,
                                scalar2=None, op0=AX.mult)
        e_sb = epool.tile([G, NCLS], F32, tag="esm")
        ss_sb = epool.tile([G, 1], F32, tag="ss")
        nc.scalar.activation(out=e_sb[:], in_=lg_sb[:],
                             func=mybir.ActivationFunctionType.Exp,
                             bias=nm_sb[:, 0:1], accum_out=ss_sb[:, 0:1])
        ls_sb = epool.tile([G, 1], F32, tag="ls")
        nc.scalar.activation(out=ls_sb[:], in_=ss_sb[:],
                             func=mybir.ActivationFunctionType.Ln)
        lsm_sb = epool.tile([G, NCLS], F32, tag="lsm")
        nc.vector.tensor_scalar(out=lsm_sb[:], in0=lg_sb[:],
                                scalar1=m_sb[:, 0:1], scalar2=ls_sb[:, 0:1],
                                op0=AX.subtract, op1=AX.subtract)

        nc.sync.dma_start(out=d_lsm[:, :], in_=lsm_sb[:])
        nc.sync.dma_start(out=d_logit[:, :], in_=lg_sb[:])

    nc.compile()  # bacc register allocation / DCE / act-table loads
    return nc


def run_gat(inputs, cfg, trace=False):
    meta, in_maps = host_prep(inputs, cfg)
    nc = build_nc(meta)
    res = run_bass_kernel_spmd(nc, in_maps, core_ids=list(range(NCORES)),
                               trace=trace)
    r0 = res.results[0]
    return (r0["out_lsm"], r0["out_logits"]), res


def kernel(**inputs):
    (lsm, logits), _ = run_gat(inputs, gat_config())
    return lsm.astype(np.float32), logits.astype(np.float32)
